# revision 1
# baseline (speedup 1.0000x reference)
"""MoE routing kernel (nn_DecFCSwitch) for 8 Trainium2 NeuronCores.

Reference computes all 16 expert branches for every token and then
selects one per token.  Only the selected branch matters, so:

  host:   sort tokens by expert, pad each expert's tokens to capacity C,
          relu(x) (the residual add also stays on host), transpose so
          the feature dim lands on SBUF partitions, cast to bf16.
  device: expert-parallel SPMD — core i owns experts {2i, 2i+1} and runs
          a 2-layer MLP (no inter-layer activation) on its experts'
          tokens.  All matmuls keep tokens on the PSUM free dim, so the
          per-expert biases are plain per-partition broadcasts.
  host:   transpose back, scatter rows to token order, out = x + sel.

Compute dtype bf16 (PSUM accumulates fp32); biases/output fp32 paths.
"""

import os
import sys

import numpy as np

for _p in ("/opt/trn_rl_repo", "/root/.axon_site/_ro/trn_rl_repo"):
    if os.path.isdir(_p) and _p not in sys.path:
        sys.path.insert(0, _p)

import ml_dtypes

B, D, S, NB = 4096, 1024, 256, 16
NCORES = 8
EPC = NB // NCORES  # experts per core
KD = D // 128  # d-dim k/m tiles
KS = S // 128  # s-dim tiles

BF16 = ml_dtypes.bfloat16

_programs = {}  # C -> compiled Bacc program
LAST_RESULT = None  # BassKernelResults of the most recent run (for test.py)


N_WARM = 36  # PE warm-up matmuls (HAM p-state ramp) before real data lands


def _build_program(C):
    import concourse.mybir as mybir
    import concourse.tile as tile
    from concourse import bacc

    cdt = mybir.dt.bfloat16
    f32 = mybir.dt.float32
    ident = mybir.ActivationFunctionType.Identity

    nc = bacc.Bacc()
    hT = nc.declare_dram_parameter("hT", [KD, 128, EPC * C], cdt, isOutput=False)
    w1 = nc.declare_dram_parameter("w1", [EPC, KD, 128, S], cdt, isOutput=False)
    w2 = nc.declare_dram_parameter("w2", [EPC, KS, 128, D], cdt, isOutput=False)
    # b_in and b_out packed: bc[e, 0:KS] = b_in tiles, bc[e, KS:KS+KD] = b_out
    bc = nc.declare_dram_parameter("bc", [EPC, KS + KD, 128, 1], f32, isOutput=False)
    yT = nc.declare_dram_parameter("yT", [KD, 128, EPC * C], cdt, isOutput=True)

    HK = KD // 2  # h is loaded in two half-loads of HK d-tiles each

    with tile.TileContext(nc) as tc:
        with (
            tc.tile_pool(name="bias", bufs=1) as bias_pool,
            tc.tile_pool(name="h", bufs=1) as h_pool,
            tc.tile_pool(name="w1p", bufs=1) as w1_pool,
            tc.tile_pool(name="w2p", bufs=1) as w2_pool,
            tc.tile_pool(name="hid", bufs=4) as hid_pool,
            tc.tile_pool(name="yout", bufs=2) as y_pool,
            tc.tile_pool(name="ps1", bufs=2, space="PSUM") as ps1_pool,
            tc.tile_pool(name="ps2", bufs=6, space="PSUM") as ps2_pool,
            tc.tile_pool(name="warm", bufs=1) as warm_pool,
        ):
            # Dummy matmuls keep the PE busy from t=0 so the HAM throttle is
            # fully ramped by the time the first real operands arrive.  The
            # warm PSUM tile borrows a ps2 slot (released before layer 2).
            wz = warm_pool.tile([128, 64], cdt, tag="wz")
            nc.gpsimd.memset(wz[:], 0)
            wps = ps2_pool.tile([128, C], f32, name="wps", tag="ps")
            for _ in range(N_WARM):
                nc.tensor.matmul(
                    wps[0:64, 0:64], lhsT=wz[:, 0:64], rhs=wz[:], start=True, stop=True
                )
            # Biases ride the SWDGE (gpsimd) path: the HWDGE rings are the
            # serial resource, Pool is idle.
            NB_COL = KS + KD
            bct = bias_pool.tile([128, EPC * NB_COL], f32, tag="bc")
            nc.gpsimd.dma_start(
                out=bct[:].rearrange("p (e t) -> p e t", e=EPC),
                in_=bc[:, :, :, 0].rearrange("e t p -> p e t"),
            )

            def b1_ap(e, t):
                return bct[:, e * NB_COL + t : e * NB_COL + t + 1]

            def b2_ap(e, k):
                return bct[:, e * NB_COL + KS + k : e * NB_COL + KS + k + 1]

            # Activations: graduated chunks (1,1,2,4 d-tiles) so the PE can
            # start as soon as the first small chunks land.
            H_CHUNKS = [(0, 2), (2, 4), (4, 6), (6, 8)]
            h_pool_tiles = [
                h_pool.tile(
                    [128, (k1 - k0) * EPC * C], cdt, tag=f"h{i}", name=f"h{i}"
                )
                for i, (k0, k1) in enumerate(H_CHUNKS)
            ]

            def load_h(i):
                k0, k1 = H_CHUNKS[i]
                nc.sync.dma_start(
                    out=h_pool_tiles[i][:].rearrange("p (k n) -> p k n", k=k1 - k0),
                    in_=hT[k0:k1].rearrange("k p n -> p k n"),
                )

            def h_slice(k, e):  # rhs [128, C] for d-tile k, expert e
                for i, (k0, k1) in enumerate(H_CHUNKS):
                    if k0 <= k < k1:
                        return h_pool_tiles[i][
                            :, ((k - k0) * EPC + e) * C : ((k - k0) * EPC + e) * C + C
                        ]
                raise AssertionError(k)

            def make_w1(e):
                return w1_pool.tile([128, KD * S], cdt, tag=f"w1_{e}", name=f"w1_{e}")

            def load_w1_part(e, w1t, k0, k1, eng=None):
                (eng or nc.sync).dma_start(
                    out=w1t[:, k0 * S : k1 * S].rearrange("p (k s) -> p k s", k=k1 - k0),
                    in_=w1[e][k0:k1].rearrange("k p s -> p k s"),
                )

            # w2 tile free layout: (q, t, d_within_quarter) — a d-quarter can
            # be loaded on its own so layer 2's m-groups unblock pairwise.
            DQ = D // 4

            def load_w2(e, w2t, eng=None):
                (eng or nc.sync).dma_start(
                    out=w2t[:].rearrange("p (q t d) -> p q t d", q=4, t=KS),
                    in_=w2[e].rearrange("t p (q d) -> p q t d", q=4),
                )

            def load_w2_quarter(e, w2t, q):
                nc.sync.dma_start(
                    out=w2t[:, q * KS * DQ : (q + 1) * KS * DQ].rearrange(
                        "p (t d) -> p t d", t=KS
                    ),
                    in_=w2[e][:, :, q * DQ : (q + 1) * DQ].rearrange("t p d -> p t d"),
                )

            def w2_slice(w2t, t, m):
                q, dd = divmod(m * 128, DQ)
                base = q * KS * DQ + t * DQ + dd
                return w2t[:, base : base + 128]

            # SP ring, in first-need order, fine-grained at the start so the
            # first matmuls unblock as early as possible.
            w1_tiles = [make_w1(0), make_w1(1)]
            w2_tiles = [
                w2_pool.tile([128, KS * D], cdt, tag=f"w2_{e}", name=f"w2_{e}")
                for e in range(EPC)
            ]
            load_h(0)
            load_w1_part(0, w1_tiles[0], 0, 4)
            load_h(1)
            load_h(2)
            load_w1_part(0, w1_tiles[0], 4, 8)
            load_h(3)

            for e in range(EPC):
                # Layer 1: hid^T[s, c] = sum_d W_in[s, d] * h^T[d, c]
                hids = []
                for t in range(KS):
                    ps = ps1_pool.tile([128, C], f32)
                    for k in range(KD):
                        nc.tensor.matmul(
                            ps[:],
                            lhsT=w1_tiles[e][:, k * S + t * 128 : k * S + t * 128 + 128],
                            rhs=h_slice(k, e),
                            start=(k == 0),
                            stop=(k == KD - 1),
                        )
                    if e == 0 and t == 0:
                        load_w2(0, w2_tiles[0])
                    elif e == 0 and t == 1:
                        load_w1_part(1, w1_tiles[1], 0, 4)
                        load_w1_part(1, w1_tiles[1], 4, 8)
                    hid = hid_pool.tile([128, C], cdt)
                    nc.scalar.activation(hid[:], ps[:], ident, bias=b1_ap(e, t))
                    hids.append(hid)

                # Layer 2: y^T[d, c] = sum_s W_out[d, s] * hid^T[s, c]
                # Evictions alternate ACT / DVE into one [128, KD*C] tile;
                # stored in two strided half-DMAs so the tail store is short.
                y_big = y_pool.tile([128, KD * C], cdt)
                for m in range(KD):
                    ps = ps2_pool.tile([128, C], f32)
                    for t in range(KS):
                        nc.tensor.matmul(
                            ps[:],
                            lhsT=w2_slice(w2_tiles[e], t, m),
                            rhs=hids[t][:],
                            start=(t == 0),
                            stop=(t == KS - 1),
                        )
                    if e == 0 and m == 0:
                        load_w2_quarter(1, w2_tiles[1], 0)
                        load_w2_quarter(1, w2_tiles[1], 1)
                    elif e == 0 and m == 2:
                        load_w2_quarter(1, w2_tiles[1], 2)
                        load_w2_quarter(1, w2_tiles[1], 3)
                    dst = y_big[:, m * C : (m + 1) * C]
                    bias_ap = b2_ap(e, m)
                    if m % 2 == 0:
                        nc.scalar.activation(dst, ps[:], ident, bias=bias_ap)
                    else:
                        nc.vector.tensor_scalar_add(dst, ps[:], bias_ap)
                    # Stores alternate between the two idle DMA issuers —
                    # gpsimd (SWDGE) and SP (HWDGE, free once loads are done) —
                    # so tail stores don't serialize on one generator.
                    store_after = {3: (0, 4), 7: (4, 8)} if e == 0 else {
                        1: (0, 2), 3: (2, 4), 5: (4, 6), 7: (6, 8)
                    }
                    if m in store_after:
                        k0, k1 = store_after[m]
                        issuer = nc.gpsimd if (m // 2) % 2 == 0 else nc.sync
                        issuer.dma_start(
                            out=yT[k0:k1, :, e * C : (e + 1) * C]
                            .rearrange("k p n -> p k n"),
                            in_=y_big[:, k0 * C : k1 * C]
                            .rearrange("p (k n) -> p k n", k=k1 - k0),
                        )

    nc.compile()
    return nc


def kernel(x, y_index, W_in, b_in, W_out, b_out):
    global LAST_RESULT
    from concourse.bass_utils import run_bass_kernel_spmd

    x = np.asarray(x, dtype=np.float32)
    W_in = np.asarray(W_in, dtype=np.float32)
    b_in = np.asarray(b_in, dtype=np.float32)
    W_out = np.asarray(W_out, dtype=np.float32)
    b_out = np.asarray(b_out, dtype=np.float32)
    eidx = np.asarray(y_index).reshape(-1).astype(np.int64)

    counts = np.bincount(eidx, minlength=NB)
    C = max(276, int(-(-counts.max() // 4) * 4))  # capacity per expert

    if C > 512:
        # Extreme expert skew would overflow a PSUM bank (512 f32 free dim);
        # fall back to exact host math rather than ship a broken program.
        out = np.empty_like(x)
        h_full = np.maximum(x, 0.0)
        for e in range(NB):
            m = eidx == e
            if m.any():
                hid = h_full[m] @ W_in[e].T + b_in[e]
                out[m] = x[m] + hid @ W_out[e].T + b_out[e]
        return out

    # --- host dispatch: group tokens by expert ---------------------------
    order = np.argsort(eidx, kind="stable")
    starts = np.zeros(NB + 1, dtype=np.int64)
    np.cumsum(counts, out=starts[1:])

    h = np.maximum(x, 0.0)
    Xg = np.zeros((NB, C, D), dtype=np.float32)
    for e in range(NB):
        toks = order[starts[e] : starts[e + 1]]
        Xg[e, : counts[e]] = h[toks]

    # [NB, C, D] -> per core [D, EPC*C] -> [KD, 128, EPC*C]
    hT_all = (
        Xg.reshape(NCORES, EPC * C, D)
        .transpose(0, 2, 1)
        .reshape(NCORES, KD, 128, EPC * C)
        .astype(BF16)
    )
    w1_all = (
        W_in.transpose(0, 2, 1).reshape(NCORES, EPC, KD, 128, S).astype(BF16)
    )
    w2_all = (
        W_out.transpose(0, 2, 1).reshape(NCORES, EPC, KS, 128, D).astype(BF16)
    )
    bc_all = np.concatenate(
        [b_in.reshape(NB, KS, 128, 1), b_out.reshape(NB, KD, 128, 1)], axis=1
    ).reshape(NCORES, EPC, KS + KD, 128, 1)

    if C not in _programs:
        _programs[C] = _build_program(C)
    nc = _programs[C]

    in_maps = [
        {
            "hT": np.ascontiguousarray(hT_all[i]),
            "w1": np.ascontiguousarray(w1_all[i]),
            "w2": np.ascontiguousarray(w2_all[i]),
            "bc": np.ascontiguousarray(bc_all[i]),
        }
        for i in range(NCORES)
    ]

    trace = bool(int(os.environ.get("KERNEL_TRACE", "0")))
    res = run_bass_kernel_spmd(nc, in_maps, list(range(NCORES)), trace=trace)
    LAST_RESULT = res

    # --- host gather: transpose back, scatter to token order -------------
    out = np.empty_like(x)
    Yg = np.stack(
        [
            r["yT"].reshape(D, EPC * C).astype(np.float32)
            for r in res.results
        ]
    )  # [NCORES, D, EPC*C]
    Yg = Yg.transpose(0, 2, 1).reshape(NB, C, D)
    for e in range(NB):
        toks = order[starts[e] : starts[e + 1]]
        out[toks] = x[toks] + Yg[e, : counts[e]]
    return out



# revision 3
# speedup vs baseline: 1.4355x; 1.4355x over previous
"""MoE routing kernel (nn_DecFCSwitch) for 8 Trainium2 NeuronCores.

Reference computes all 16 expert branches for every token and then
selects one per token.  Only the selected branch matters, so:

  host:   sort tokens by expert, pad each expert's tokens to capacity C,
          relu(x), cast to fp8e4m3 (weights pre-scaled by 256 so all
          values sit in e4m3's normal range), lay every tensor out
          partition-major so each DMA is one big contiguous run per
          partition (full 360 B/ns bus, no <512B penalty).
  device: expert-parallel SPMD - core i owns experts {2i, 2i+1}.
          Both layers run as fp8 DoubleRow matmuls (256-deep
          contraction, 0.5 cycles/row).  PSUM evictions fuse the
          1/256 de-scale + f32 bias and cast to fp8: ACT takes even
          tiles, DVE odd tiles.  Stores stream out in 4-tile chunks on
          the idle SWDGE path; the last small chunk rides HWDGE.
  host:   decode fp8 -> f32, scatter rows to token order, out = x + sel.
"""

import os
import sys

import numpy as np

for _p in ("/opt/trn_rl_repo", "/root/.axon_site/_ro/trn_rl_repo"):
    if os.path.isdir(_p) and _p not in sys.path:
        sys.path.insert(0, _p)

import ml_dtypes

B, D, S, NB = 4096, 1024, 256, 16
NCORES = 8
EPC = NB // NCORES  # experts per core
KD = D // 128  # d-dim 128-tiles
KS = S // 128  # s-dim 128-tiles
K2 = D // 256  # d-dim 256-tiles (DoubleRow contraction chunks)

F8 = ml_dtypes.float8_e4m3
WSCALE = 256.0  # host-side weight scale (power of two; undone at eviction)

_programs = {}  # C -> compiled Bacc program
LAST_RESULT = None  # BassKernelResults of the most recent run (for test.py)

# Warm-matmul counts per slot: bridge PE busy time through the DMA-bound
# stretch so the HAM p-state is fully ramped when the tail phases run.
WARM = (7, 2, 2, 2, 2)


def _build_program(C):
    import concourse.mybir as mybir
    import concourse.tile as tile
    from concourse import bacc

    f8 = mybir.dt.float8e4
    bf16 = mybir.dt.bfloat16
    f32 = mybir.dt.float32
    ident = mybir.ActivationFunctionType.Identity
    DR = mybir.MatmulPerfMode.DoubleRow
    mul_op = mybir.AluOpType.mult
    add_op = mybir.AluOpType.add

    nc = bacc.Bacc()
    # All DRAM params partition-major: [128, cols], cols contiguous per row.
    hT = nc.declare_dram_parameter("hT", [128, KD * EPC * C], f8, isOutput=False)
    w1 = nc.declare_dram_parameter("w1", [128, EPC * 2048], f8, isOutput=False)
    w2 = nc.declare_dram_parameter("w2", [128, EPC * 2048], f8, isOutput=False)
    bc = nc.declare_dram_parameter("bc", [128, EPC * (KS + KD)], f32, isOutput=False)
    yT = nc.declare_dram_parameter("yT", [128, EPC * KD * C], f8, isOutput=True)

    NBC = KS + KD  # bias cols per expert
    HC = KD * EPC * C // 2  # h half-load cols

    with tile.TileContext(nc) as tc:
        with (
            tc.tile_pool(name="bias", bufs=1) as bias_pool,
            tc.tile_pool(name="h", bufs=1) as h_pool,
            tc.tile_pool(name="w1p", bufs=1) as w1_pool,
            tc.tile_pool(name="w2p", bufs=1) as w2_pool,
            tc.tile_pool(name="hid", bufs=2) as hid_pool,
            tc.tile_pool(name="yout", bufs=2) as y_pool,
            tc.tile_pool(name="ps", bufs=7, space="PSUM") as ps_pool,
            tc.tile_pool(name="warm", bufs=1) as warm_pool,
            tc.tile_pool(name="wps", bufs=1, space="PSUM") as wps_pool,
        ):
            # --- setup: warm tile + bias ride the idle DVE / SWDGE paths ---
            wz = warm_pool.tile([128, 512], bf16, tag="wz")
            nc.vector.memset(wz[:], 0)
            wps = wps_pool.tile([128, 512], f32, name="wps", tag="wps")

            bct = bias_pool.tile([128, EPC * NBC], f32, tag="bc")
            nc.gpsimd.dma_start(out=bct[:], in_=bc[:, :])

            def b1_ap(e, t):
                return bct[:, e * NBC + t : e * NBC + t + 1]

            def b2_ap(e, m):
                return bct[:, e * NBC + KS + m : e * NBC + KS + m + 1]

            # --- SBUF tiles -------------------------------------------------
            ht = h_pool.tile([128, KD * EPC * C], f8, tag="h")
            w1t = w1_pool.tile([128, EPC * 2048], f8, tag="w1")
            w2t = w2_pool.tile([128, EPC * 2048], f8, tag="w2")
            hid = [
                hid_pool.tile([128, KS * C], f8, tag=f"hid{e}", name=f"hid{e}")
                for e in range(EPC)
            ]
            ybig = [
                y_pool.tile([128, KD * C], f8, tag=f"y{e}", name=f"y{e}")
                for e in range(EPC)
            ]

            # --- loads (HWDGE on SP), in first-use order --------------------
            def load_cols(dst, src, c0, c1):
                nc.sync.dma_start(out=dst[:, c0:c1], in_=src[:, c0:c1])

            load_cols(w1t, w1, 0, 2048)  # w1[e0]
            load_cols(ht, hT, 0, HC)  # h first half (k 0..3)
            load_cols(ht, hT, HC, 2 * HC)  # h second half (k 4..7)
            load_cols(w2t, w2, 0, 2048)  # w2[e0]
            load_cols(w1t, w1, 2048, 4096)  # w1[e1]
            load_cols(w2t, w2, 2048, 2048 + 1024)  # w2[e1] m0-3
            load_cols(w2t, w2, 2048 + 1024, 4096)  # w2[e1] m4-7

            ht_v = ht[:].rearrange("p (k e c) -> p k e c", k=KD, e=EPC)

            def warm_block(n):
                for _ in range(n):
                    nc.tensor.matmul(
                        wps[:], lhsT=wz[:, 0:128], rhs=wz[:], start=True, stop=True
                    )

            def l1_matmul(e, t, k2, ps):
                base = ((e * KS + t) * K2 + k2) * 256
                nc.tensor.matmul(
                    ps[:],
                    lhsT=w1t[:, base : base + 256].rearrange("p (i m) -> p i m", i=2),
                    rhs=ht_v[:, 2 * k2 : 2 * k2 + 2, e],
                    start=(k2 == 0),
                    stop=(k2 == K2 - 1),
                    perf_mode=DR,
                )

            def evict(eng_idx, dst, ps, bias):
                # out = ps/WSCALE + bias, cast to fp8
                if eng_idx == 0:
                    nc.scalar.activation(dst, ps[:], ident, bias=bias, scale=1.0 / WSCALE)
                else:
                    nc.vector.tensor_scalar(
                        dst, ps[:], 1.0 / WSCALE, bias, mul_op, add_op
                    )

            warm_block(WARM[0])

            # --- expert 0 ---------------------------------------------------
            # L1: hid[s,c] accumulated over 4 DoubleRow chunks of d
            for e in range(EPC):
                ps1 = []
                for t in range(KS):
                    ps = ps_pool.tile([128, C], f32, name="ps")
                    for k2 in range(K2):
                        l1_matmul(e, t, k2, ps)
                    ps1.append(ps)
                for t in range(KS):
                    evict(t % 2, hid[e][:, t * C : (t + 1) * C], ps1[t], b1_ap(e, t))

                if e == 0:
                    warm_block(WARM[1])

                # L2: y[d,c] - one DoubleRow matmul per 128-row output tile
                hid_v = hid[e][:].rearrange("p (i c) -> p i c", i=KS)
                for m in range(KD):
                    ps = ps_pool.tile([128, C], f32, name="ps")
                    base = (e * KD + m) * 256
                    nc.tensor.matmul(
                        ps[:],
                        lhsT=w2t[:, base : base + 256].rearrange(
                            "p (i m) -> p i m", i=2
                        ),
                        rhs=hid_v,
                        start=True,
                        stop=True,
                        perf_mode=DR,
                    )
                    evict(m % 2, ybig[e][:, m * C : (m + 1) * C], ps, b2_ap(e, m))
                    if m == 3:
                        # first half-store on the idle SWDGE path
                        nc.gpsimd.dma_start(
                            out=yT[:, e * KD * C : e * KD * C + 4 * C],
                            in_=ybig[e][:, 0 : 4 * C],
                        )
                    elif m == KD - 1:
                        issuer = nc.gpsimd if e == 0 else nc.sync
                        issuer.dma_start(
                            out=yT[:, e * KD * C + 4 * C : (e + 1) * KD * C],
                            in_=ybig[e][:, 4 * C : KD * C],
                        )
                if e == 0:
                    warm_block(WARM[2])

    nc.compile()
    return nc


def kernel(x, y_index, W_in, b_in, W_out, b_out):
    global LAST_RESULT
    from concourse.bass_utils import run_bass_kernel_spmd

    x = np.asarray(x, dtype=np.float32)
    W_in = np.asarray(W_in, dtype=np.float32)
    b_in = np.asarray(b_in, dtype=np.float32)
    W_out = np.asarray(W_out, dtype=np.float32)
    b_out = np.asarray(b_out, dtype=np.float32)
    eidx = np.asarray(y_index).reshape(-1).astype(np.int64)

    counts = np.bincount(eidx, minlength=NB)
    C = max(276, int(-(-counts.max() // 4) * 4))  # capacity per expert

    if C > 512:
        # Extreme expert skew would overflow a PSUM bank (512 f32 free dim);
        # fall back to exact host math rather than ship a broken program.
        out = np.empty_like(x)
        h_full = np.maximum(x, 0.0)
        for e in range(NB):
            m = eidx == e
            if m.any():
                hid = h_full[m] @ W_in[e].T + b_in[e]
                out[m] = x[m] + hid @ W_out[e].T + b_out[e]
        return out

    # --- host dispatch: group tokens by expert ---------------------------
    order = np.argsort(eidx, kind="stable")
    starts = np.zeros(NB + 1, dtype=np.int64)
    np.cumsum(counts, out=starts[1:])

    h = np.maximum(x, 0.0)
    Xg = np.zeros((NB, C, D), dtype=np.float32)
    for e in range(NB):
        toks = order[starts[e] : starts[e + 1]]
        Xg[e, : counts[e]] = h[toks]

    # hT: [core, 128, (k, e, c)] - value = h[token (e,c), 128k + p]
    hT_all = np.ascontiguousarray(
        Xg.astype(F8)
        .reshape(NCORES, EPC, C, KD, 128)
        .transpose(0, 4, 3, 1, 2)
        .reshape(NCORES, 128, KD * EPC * C)
    )
    # w1: [core, 128, (e, t, k2, i, m)] = W_in[e, 128t+m, 256k2+128i+p] * 256
    w1_all = np.ascontiguousarray(
        (W_in * WSCALE)
        .astype(F8)
        .reshape(NCORES, EPC, KS, 128, K2, 2, 128)
        .transpose(0, 6, 1, 2, 4, 5, 3)
        .reshape(NCORES, 128, EPC * 2048)
    )
    # w2: [core, 128, (e, m, i, j)] = W_out[e, 128m+j, 128i+p] * 256
    w2_all = np.ascontiguousarray(
        (W_out * WSCALE)
        .astype(F8)
        .reshape(NCORES, EPC, KD, 128, KS, 128)
        .transpose(0, 5, 1, 2, 4, 3)
        .reshape(NCORES, 128, EPC * 2048)
    )
    # bc: [core, 128, (e, {t than m})] f32
    bc_all = np.ascontiguousarray(
        np.concatenate(
            [b_in.reshape(NB, KS, 128), b_out.reshape(NB, KD, 128)], axis=1
        )
        .reshape(NCORES, EPC, KS + KD, 128)
        .transpose(0, 3, 1, 2)
        .reshape(NCORES, 128, EPC * (KS + KD))
    )

    if C not in _programs:
        _programs[C] = _build_program(C)
    nc = _programs[C]

    in_maps = [
        {
            "hT": hT_all[i],
            "w1": w1_all[i],
            "w2": w2_all[i],
            "bc": bc_all[i],
        }
        for i in range(NCORES)
    ]

    trace = bool(int(os.environ.get("KERNEL_TRACE", "0")))
    res = run_bass_kernel_spmd(nc, in_maps, list(range(NCORES)), trace=trace)
    LAST_RESULT = res

    # --- host gather: decode fp8, scatter to token order -----------------
    out = np.empty_like(x)
    Yg = np.stack(
        [np.asarray(r["yT"]).astype(np.float32) for r in res.results]
    )  # [NCORES, 128, EPC*KD*C]
    Yg = (
        Yg.reshape(NCORES, 128, EPC, KD, C)
        .transpose(0, 2, 4, 3, 1)
        .reshape(NB, C, D)
    )
    for e in range(NB):
        toks = order[starts[e] : starts[e + 1]]
        out[toks] = x[toks] + Yg[e, : counts[e]]
    return out
